# revision 31
# baseline (speedup 1.0000x reference)
"""Distributed brute-force KNN (retrieval) kernel for one TRN2 chip (8 NeuronCores).

Problem: queries [256,128] f32, candidates [500000,128] f32, identifiers [500000] i32,
k=100. Output: (values [256,100] f32 desc-sorted, ids [256,100] i32).

Strategy (v2 — group-max fold, no on-device extraction):
  - Shard candidates over N across the 8 cores (62500 each, zero-padded to
    63488 = 31 chunks x 2048).
  - Per core: bf16 matmul (Q stationary) -> PSUM f32 score chunks
    [128q, 2048c] (4 banks). Each chunk is folded by pairwise max down
    to 2048/FOLD bf16 group-maxima (member j of slot i is local col
    chunk*2048 + i + (2048/FOLD)*j). Fold1 mixes a PSUM operand with a
    ScalarE-evacuated SBUF operand (flows A/C balance ACT vs DVE). All
    slots per query-half accumulate in SBUF; one DMA out per half.
  - Host: rescore the top-C claimed groups exactly in f64, derive the
    device claim error bound, extend the selection to every group whose
    claim could still reach top-k, and take the exact top-k. Exactness
    never depends on device numerics (groups cover ALL candidates).
"""
import numpy as np
import ml_dtypes

B = 256          # queries
N = 500000       # candidates
D = 128          # dim
NCORES = 8
NSH = N // NCORES          # 62500 real candidates per core
CHUNK = 2048               # candidates per fold unit (4 PSUM banks)
NCHUNK = 31                # chunks per core
NSHP = NCHUNK * CHUNK      # 63488 padded candidates per core
FOLD = 4                   # candidates per claimed slot
SLOTS = CHUNK // FOLD      # slots per chunk
NSLOT = NCHUNK * SLOTS     # slots per (core, query)

_CACHE = {}


def build(loops=1, variant="mix", amod=15, athr=7, cbufs=6):
    """Build + compile the per-core Bass program. Returns the compiled Bacc.

    Two evacuation flows per 2048-chunk (TT with both operands in PSUM is
    an ISA violation, so fold1 always has >=1 SBUF operand):
      A: ScalarE copies all 2048 f32 PSUM -> SBUF bf16; VectorE does 4
         bf16 folds (2048->128) at 2x.
      C: ScalarE copies cols [1024:2048] only; VectorE fold1 is a mixed
         TT (PSUM f32 x SBUF bf16 -> bf16, 1x), then 3 bf16 folds.
    variant "mix": unit u is flow A iff (u % amod) < athr (default 7/12
    ~ the ACT/DVE balance point); "allact"/"allc" force one flow.
    """
    import concourse.bass as bass
    import concourse.tile as tile
    from concourse import bacc, mybir

    bf16 = mybir.dt.bfloat16
    f32 = mybir.dt.float32
    Copy = mybir.ActivationFunctionType.Copy

    nc = bacc.Bacc("TRN2", debug=False)
    qt = nc.dram_tensor("qt", [D, B], bf16, kind="ExternalInput").ap()
    ct = nc.dram_tensor("ct", [NCHUNK, D, CHUNK], bf16, kind="ExternalInput").ap()
    v8 = nc.dram_tensor("v8", [B, NSLOT], bf16, kind="ExternalOutput").ap()

    def flow_of(u):
        if variant == "allact":
            return "A"
        if variant == "allc" or variant == "3way":
            return "C"
        return "A" if (u % amod) < athr else "C"

    with tile.TileContext(nc) as tc:
        with (
            tc.tile_pool(name="qpool", bufs=1) as qpool,
            tc.tile_pool(name="cpool", bufs=cbufs) as cpool,
            tc.tile_pool(name="psum", bufs=2, space="PSUM") as pp,
            tc.tile_pool(name="evac", bufs=3) as epool,
            tc.tile_pool(name="fold", bufs=3) as fpool,
            tc.tile_pool(name="acc", bufs=1) as accp,
        ):
            qtile = qpool.tile([D, B], bf16)
            nc.sync.dma_start(qtile[:], qt[:])
            vacc = [
                accp.tile([128, NSLOT], bf16, tag=f"vacc{h}", name=f"vacc{h}")
                for h in range(2)
            ]

            def body(_iv=None):
                u = 0
                for c in range(NCHUNK):
                    ctile = cpool.tile([D, CHUNK], bf16, tag="ct", name="ctile")
                    if variant in ("halfdma", "nothing2"):
                        nc.sync.dma_start(
                            ctile[:, bass.ds(0, 1024)],
                            ct[c, :, bass.ds(0, 1024)],
                        )
                    elif variant == "nothing4":
                        nc.sync.dma_start(
                            ctile[:, bass.ds(0, 1024)], ct[c, :, bass.ds(0, 1024)]
                        )
                        nc.sync.dma_start(
                            ctile[:, bass.ds(1024, 1024)],
                            ct[c, :, bass.ds(1024, 1024)],
                        )
                    else:
                        nc.sync.dma_start(ctile[:], ct[c, :, :])
                    for h in range(2):
                        ps = pp.tile([128, CHUNK], f32, name="ps")
                        nmm = 8 if variant in ("doublepe", "nothing3") else 4
                        for j in range(nmm):
                            rj = (j % 2) if variant in ("halfdma", "nothing2") else (j % 4)
                            nc.tensor.matmul(
                                ps[:, bass.ds((j % 4) * 512, 512)],
                                lhsT=qtile[:, bass.ds(h * 128, 128)],
                                rhs=ctile[:, bass.ds(rj * 512, 512)],
                                start=True,
                                stop=True,
                            )
                        if variant.startswith("nothing"):
                            u += 1
                            continue
                        if variant == "folds0":
                            sc = epool.tile([128, CHUNK], bf16, tag="sc", name="sc")
                            nc.scalar.activation(sc[:], ps[:], Copy)
                            u += 1
                            continue
                        f1 = fpool.tile([128, 1024], bf16, tag="f1", name="f1")
                        if variant == "3way" and (u % amod) < athr:
                            sc = epool.tile([128, CHUNK], bf16, tag="sc", name="sc")
                            nc.scalar.activation(sc[:], ps[:], Copy)
                            nc.gpsimd.tensor_max(
                                f1[:],
                                sc[:, bass.ds(0, 1024)],
                                sc[:, bass.ds(1024, 1024)],
                            )
                            w = 1024
                            cur = f1
                            while w // 2 > SLOTS:
                                w //= 2
                                nxt = fpool.tile(
                                    [128, w], bf16, tag=f"f{w}", name=f"f{w}"
                                )
                                nc.gpsimd.tensor_max(
                                    nxt[:],
                                    cur[:, bass.ds(0, w)],
                                    cur[:, bass.ds(w, w)],
                                )
                                cur = nxt
                            nc.gpsimd.tensor_max(
                                vacc[h][:, bass.ds(c * SLOTS, SLOTS)],
                                cur[:, bass.ds(0, SLOTS)],
                                cur[:, bass.ds(SLOTS, SLOTS)],
                            )
                            u += 1
                            continue
                        if flow_of(u) == "A":
                            sc = epool.tile([128, CHUNK], bf16, tag="sc", name="sc")
                            nc.scalar.activation(sc[:], ps[:], Copy)
                            nc.vector.tensor_max(
                                f1[:],
                                sc[:, bass.ds(0, 1024)],
                                sc[:, bass.ds(1024, 1024)],
                            )
                        else:
                            sc = epool.tile([128, 1024], bf16, tag="sc2", name="sc2")
                            nc.scalar.activation(sc[:], ps[:, bass.ds(1024, 1024)], Copy)
                            nc.vector.tensor_max(
                                f1[:], ps[:, bass.ds(0, 1024)], sc[:]
                            )
                        # remaining bf16 folds down to SLOTS wide
                        w = 1024
                        cur = f1
                        while w // 2 > SLOTS:
                            w //= 2
                            nxt = fpool.tile([128, w], bf16, tag=f"f{w}", name=f"f{w}")
                            nc.vector.tensor_max(
                                nxt[:], cur[:, bass.ds(0, w)], cur[:, bass.ds(w, w)]
                            )
                            cur = nxt
                        nc.vector.tensor_max(
                            vacc[h][:, bass.ds(c * SLOTS, SLOTS)],
                            cur[:, bass.ds(0, SLOTS)],
                            cur[:, bass.ds(SLOTS, SLOTS)],
                        )
                        u += 1

            if loops == 1:
                body()
            else:
                with tc.For_i(0, loops, 1) as iv:
                    body(iv)

            if variant != "folds0" and not variant.startswith("nothing"):
                for h in range(2):
                    nc.sync.dma_start(v8[bass.ds(h * 128, 128), :], vacc[h][:])
    nc.compile()
    return nc


def _get_nc():
    if "nc" not in _CACHE:
        _CACHE["nc"] = build()
    return _CACHE["nc"]


def make_in_maps(queries, candidates):
    qt = np.ascontiguousarray(queries.T).astype(ml_dtypes.bfloat16)
    cb = candidates.astype(ml_dtypes.bfloat16)
    in_maps = []
    for c in range(NCORES):
        flat = np.zeros((D, NSHP), dtype=ml_dtypes.bfloat16)
        flat[:, :NSH] = cb[c * NSH : (c + 1) * NSH].T
        ct = np.ascontiguousarray(
            flat.reshape(D, NCHUNK, CHUNK).transpose(1, 0, 2)
        )
        in_maps.append({"qt": qt, "ct": ct})
    return in_maps


def _device_claims(queries, candidates):
    """Run the 8-core SPMD kernel; return claims [NCORES, B, NSLOT] f32."""
    from concourse.bass_utils import run_bass_kernel_spmd

    nc = _get_nc()
    in_maps = make_in_maps(queries, candidates)
    res = None
    for attempt in range(3):
        try:
            res = run_bass_kernel_spmd(nc, in_maps, core_ids=list(range(NCORES))).results
            break
        except Exception:
            if attempt == 2:
                raise
            import time as _time

            _time.sleep(2.0)
    assert res is not None
    return np.stack([r["v8"] for r in res]).astype(np.float32)


def kernel(queries, candidates, identifiers, k):
    queries = np.asarray(queries, dtype=np.float32)
    candidates = np.asarray(candidates, dtype=np.float32)
    identifiers = np.asarray(identifiers)
    kk = int(k)

    v8 = _device_claims(queries, candidates)            # [8, B, NSLOT]

    # flatten claims to [B, NCORES*NSLOT]; group g = (core, slotcol)
    vals = v8.transpose(1, 0, 2).reshape(B, NCORES * NSLOT)

    q64 = queries.astype(np.float64)
    sigma = np.linalg.norm(queries, axis=1)

    # group id -> member global candidate indices [..., FOLD] (or <0 invalid)
    def members_of(g):
        core, sl = g // NSLOT, g % NSLOT
        c, i = sl // SLOTS, sl % SLOTS
        L = (c * CHUNK + i)[..., None] + SLOTS * np.arange(FOLD)
        valid = L < NSH
        gl = L + (core * NSH)[..., None]
        return np.where(valid, gl, -1)

    def rescore(mem, qidx):
        """mem [Q, M, FOLD] global ids (-1 invalid) -> exact f64 scores."""
        Q = mem.shape[0]
        out = np.empty(mem.shape, np.float64)
        step = 64
        for s in range(0, Q, step):
            e = min(s + step, Q)
            blk = mem[s:e]
            safe = np.where(blk >= 0, blk, 0)
            sv = np.einsum(
                "qmfd,qd->qmf",
                candidates[safe].astype(np.float64),
                q64[qidx[s:e]],
            )
            out[s:e] = np.where(blk >= 0, sv, -np.inf)
        return out

    # --- preselect top-C groups per query, rescore exactly ---
    C = max(2 * kk, kk + 64)
    part = np.argpartition(-vals, C, axis=1)[:, :C]
    vsel = np.take_along_axis(vals, part, 1)
    mem = members_of(part)                              # [B, C, FOLD]
    allq = np.arange(B)
    se = rescore(mem, allq)                             # [B, C, FOLD]
    gmax = se.max(2)
    finite = np.isfinite(gmax)
    delta = np.where(finite, np.abs(vsel - gmax), 0.0).max(1)
    margin = 4.0 * delta + 1e-3 * sigma

    flat = se.reshape(B, -1)
    vk = -np.partition(-flat, kk - 1, axis=1)[:, kk - 1]
    thr = vk - margin

    pool_v = [flat[q] for q in range(B)]
    pool_g = [mem[q].reshape(-1) for q in range(B)]

    # any group above thr that wasn't rescored yet
    selmask = np.zeros(vals.shape, dtype=bool)
    np.put_along_axis(selmask, part, True, 1)
    need = (vals >= thr[:, None]) & ~selmask
    for q in np.nonzero(need.any(1))[0]:
        g = np.nonzero(need[q])[0]
        m = members_of(g)[None]                          # [1, M, FOLD]
        sv = rescore(m, np.array([q]))[0]
        pool_v[q] = np.concatenate([pool_v[q], sv.reshape(-1)])
        pool_g[q] = np.concatenate([pool_g[q], m[0].reshape(-1)])

    # --- final exact top-k per query (dedupe, desc value, index tiebreak) --
    out_v = np.empty((B, kk), np.float32)
    out_g = np.empty((B, kk), np.int64)
    for q in range(B):
        keep = pool_g[q] >= 0
        g, first = np.unique(pool_g[q][keep], return_index=True)
        v32 = pool_v[q][keep][first].astype(np.float32)
        assert v32.size >= kk
        order = np.lexsort((g, -v32))[:kk]
        out_v[q] = v32[order]
        out_g[q] = g[order]

    top_ids = identifiers[out_g]
    return out_v, top_ids


# revision 33
# speedup vs baseline: 1.1902x; 1.1902x over previous
"""Distributed brute-force KNN (retrieval) kernel for one TRN2 chip (8 NeuronCores).

Problem: queries [256,128] f32, candidates [500000,128] f32, identifiers [500000] i32,
k=100. Output: (values [256,100] f32 desc-sorted, ids [256,100] i32).

Strategy (v2 — group-max fold, no on-device extraction):
  - Shard candidates over N across the 8 cores (62500 each, zero-padded to
    63488 = 31 chunks x 2048).
  - Per core: bf16 matmul (Q stationary) -> PSUM f32 score chunks
    [128q, 2048c] (4 banks). Each chunk is folded by pairwise max down
    to 2048/FOLD bf16 group-maxima (member j of slot i is local col
    chunk*2048 + i + (2048/FOLD)*j). Fold1 mixes a PSUM operand with a
    ScalarE-evacuated SBUF operand (flows A/C balance ACT vs DVE). All
    slots per query-half accumulate in SBUF; one DMA out per half.
  - Host: rescore the top-C claimed groups exactly in f64, derive the
    device claim error bound, extend the selection to every group whose
    claim could still reach top-k, and take the exact top-k. Exactness
    never depends on device numerics (groups cover ALL candidates).
"""
import numpy as np
import ml_dtypes

B = 256          # queries
N = 500000       # candidates
D = 128          # dim
NCORES = 8
NSH = N // NCORES          # 62500 real candidates per core
CHUNK = 1024               # candidates per fold unit (2 PSUM banks)
NCHUNK = 62                # chunks per core
NSHP = NCHUNK * CHUNK      # 63488 padded candidates per core
FOLD = 4                   # candidates per claimed slot
SLOTS = CHUNK // FOLD      # slots per chunk
NSLOT = NCHUNK * SLOTS     # slots per (core, query)

_CACHE = {}


def build(loops=1, variant="mix", amod=15, athr=7, cbufs=6):
    """Build + compile the per-core Bass program. Returns the compiled Bacc.

    Two evacuation flows per 2048-chunk (TT with both operands in PSUM is
    an ISA violation, so fold1 always has >=1 SBUF operand):
      A: ScalarE copies all 2048 f32 PSUM -> SBUF bf16; VectorE does 4
         bf16 folds (2048->128) at 2x.
      C: ScalarE copies cols [1024:2048] only; VectorE fold1 is a mixed
         TT (PSUM f32 x SBUF bf16 -> bf16, 1x), then 3 bf16 folds.
    variant "mix": unit u is flow A iff (u % amod) < athr (default 7/12
    ~ the ACT/DVE balance point); "allact"/"allc" force one flow.
    """
    import concourse.bass as bass
    import concourse.tile as tile
    from concourse import bacc, mybir

    bf16 = mybir.dt.bfloat16
    f32 = mybir.dt.float32
    Copy = mybir.ActivationFunctionType.Copy

    nc = bacc.Bacc("TRN2", debug=False)
    qt = nc.dram_tensor("qt", [D, B], bf16, kind="ExternalInput").ap()
    ct = nc.dram_tensor("ct", [NCHUNK, D, CHUNK], bf16, kind="ExternalInput").ap()
    v8 = nc.dram_tensor("v8", [B, NSLOT], bf16, kind="ExternalOutput").ap()

    def flow_of(u):
        if variant == "allact":
            return "A"
        if variant == "allc" or variant == "3way":
            return "C"
        return "A" if (u % amod) < athr else "C"

    with tile.TileContext(nc) as tc:
        with (
            tc.tile_pool(name="qpool", bufs=1) as qpool,
            tc.tile_pool(name="cpool", bufs=cbufs) as cpool,
            tc.tile_pool(name="psum", bufs=4, space="PSUM") as pp,
            tc.tile_pool(name="evac", bufs=4) as epool,
            tc.tile_pool(name="fold", bufs=4) as fpool,
            tc.tile_pool(name="acc", bufs=1) as accp,
        ):
            qtile = qpool.tile([D, B], bf16)
            nc.sync.dma_start(qtile[:], qt[:])
            vacc = [
                accp.tile([128, NSLOT], bf16, tag=f"vacc{h}", name=f"vacc{h}")
                for h in range(2)
            ]

            def body(_iv=None):
                u = 0
                for cc in range(NCHUNK // 2):
                    ctile = cpool.tile([D, 2 * CHUNK], bf16, tag="ct", name="ctile")
                    for s2 in range(2):
                        nc.sync.dma_start(
                            ctile[:, bass.ds(s2 * CHUNK, CHUNK)],
                            ct[2 * cc + s2, :, :],
                        )
                    for sub in range(2):
                      c = 2 * cc + sub
                      for h in range(2):
                        ps = pp.tile([128, CHUNK], f32, name="ps")
                        for j in range(2):
                            nc.tensor.matmul(
                                ps[:, bass.ds(j * 512, 512)],
                                lhsT=qtile[:, bass.ds(h * 128, 128)],
                                rhs=ctile[:, bass.ds(sub * CHUNK + j * 512, 512)],
                                start=True,
                                stop=True,
                            )
                        if variant.startswith("nothing"):
                            u += 1
                            continue
                        if variant == "folds0":
                            sc = epool.tile([128, CHUNK], bf16, tag="sc", name="sc")
                            nc.scalar.activation(sc[:], ps[:], Copy)
                            u += 1
                            continue
                        f1 = fpool.tile([128, CHUNK // 2], bf16, tag="f1", name="f1")
                        if variant == "3way" and (u % amod) < athr:
                            sc = epool.tile([128, CHUNK], bf16, tag="sc", name="sc")
                            nc.scalar.activation(sc[:], ps[:], Copy)
                            nc.gpsimd.tensor_max(
                                f1[:],
                                sc[:, bass.ds(0, 1024)],
                                sc[:, bass.ds(1024, 1024)],
                            )
                            w = 1024
                            cur = f1
                            while w // 2 > SLOTS:
                                w //= 2
                                nxt = fpool.tile(
                                    [128, w], bf16, tag=f"f{w}", name=f"f{w}"
                                )
                                nc.gpsimd.tensor_max(
                                    nxt[:],
                                    cur[:, bass.ds(0, w)],
                                    cur[:, bass.ds(w, w)],
                                )
                                cur = nxt
                            nc.gpsimd.tensor_max(
                                vacc[h][:, bass.ds(c * SLOTS, SLOTS)],
                                cur[:, bass.ds(0, SLOTS)],
                                cur[:, bass.ds(SLOTS, SLOTS)],
                            )
                            u += 1
                            continue
                        if flow_of(u) == "A":
                            sc = epool.tile([128, CHUNK], bf16, tag="sc", name="sc")
                            nc.scalar.activation(sc[:], ps[:], Copy)
                            nc.vector.tensor_max(
                                f1[:],
                                sc[:, bass.ds(0, CHUNK // 2)],
                                sc[:, bass.ds(CHUNK // 2, CHUNK // 2)],
                            )
                        else:
                            sc = epool.tile(
                                [128, CHUNK // 2], bf16, tag="sc2", name="sc2"
                            )
                            nc.scalar.activation(
                                sc[:], ps[:, bass.ds(CHUNK // 2, CHUNK // 2)], Copy
                            )
                            nc.vector.tensor_max(
                                f1[:], ps[:, bass.ds(0, CHUNK // 2)], sc[:]
                            )
                        # remaining bf16 folds down to SLOTS wide
                        w = CHUNK // 2
                        cur = f1
                        while w // 2 > SLOTS:
                            w //= 2
                            nxt = fpool.tile([128, w], bf16, tag=f"f{w}", name=f"f{w}")
                            nc.vector.tensor_max(
                                nxt[:], cur[:, bass.ds(0, w)], cur[:, bass.ds(w, w)]
                            )
                            cur = nxt
                        nc.vector.tensor_max(
                            vacc[h][:, bass.ds(c * SLOTS, SLOTS)],
                            cur[:, bass.ds(0, SLOTS)],
                            cur[:, bass.ds(SLOTS, SLOTS)],
                        )
                        u += 1

            if loops == 1:
                body()
            else:
                with tc.For_i(0, loops, 1) as iv:
                    body(iv)

            if variant != "folds0" and not variant.startswith("nothing"):
                for h in range(2):
                    nc.sync.dma_start(v8[bass.ds(h * 128, 128), :], vacc[h][:])
    nc.compile()
    return nc


def _get_nc():
    if "nc" not in _CACHE:
        _CACHE["nc"] = build()
    return _CACHE["nc"]


def make_in_maps(queries, candidates):
    qt = np.ascontiguousarray(queries.T).astype(ml_dtypes.bfloat16)
    cb = candidates.astype(ml_dtypes.bfloat16)
    in_maps = []
    for c in range(NCORES):
        flat = np.zeros((D, NSHP), dtype=ml_dtypes.bfloat16)
        flat[:, :NSH] = cb[c * NSH : (c + 1) * NSH].T
        ct = np.ascontiguousarray(
            flat.reshape(D, NCHUNK, CHUNK).transpose(1, 0, 2)
        )
        in_maps.append({"qt": qt, "ct": ct})
    return in_maps


def _device_claims(queries, candidates):
    """Run the 8-core SPMD kernel; return claims [NCORES, B, NSLOT] f32."""
    from concourse.bass_utils import run_bass_kernel_spmd

    nc = _get_nc()
    in_maps = make_in_maps(queries, candidates)
    res = None
    for attempt in range(3):
        try:
            res = run_bass_kernel_spmd(nc, in_maps, core_ids=list(range(NCORES))).results
            break
        except Exception:
            if attempt == 2:
                raise
            import time as _time

            _time.sleep(2.0)
    assert res is not None
    return np.stack([r["v8"] for r in res]).astype(np.float32)


def kernel(queries, candidates, identifiers, k):
    queries = np.asarray(queries, dtype=np.float32)
    candidates = np.asarray(candidates, dtype=np.float32)
    identifiers = np.asarray(identifiers)
    kk = int(k)

    v8 = _device_claims(queries, candidates)            # [8, B, NSLOT]

    # flatten claims to [B, NCORES*NSLOT]; group g = (core, slotcol)
    vals = v8.transpose(1, 0, 2).reshape(B, NCORES * NSLOT)

    q64 = queries.astype(np.float64)
    sigma = np.linalg.norm(queries, axis=1)

    # group id -> member global candidate indices [..., FOLD] (or <0 invalid)
    def members_of(g):
        core, sl = g // NSLOT, g % NSLOT
        c, i = sl // SLOTS, sl % SLOTS
        L = (c * CHUNK + i)[..., None] + SLOTS * np.arange(FOLD)
        valid = L < NSH
        gl = L + (core * NSH)[..., None]
        return np.where(valid, gl, -1)

    def rescore(mem, qidx):
        """mem [Q, M, FOLD] global ids (-1 invalid) -> exact f64 scores."""
        Q = mem.shape[0]
        out = np.empty(mem.shape, np.float64)
        step = 64
        for s in range(0, Q, step):
            e = min(s + step, Q)
            blk = mem[s:e]
            safe = np.where(blk >= 0, blk, 0)
            sv = np.einsum(
                "qmfd,qd->qmf",
                candidates[safe].astype(np.float64),
                q64[qidx[s:e]],
            )
            out[s:e] = np.where(blk >= 0, sv, -np.inf)
        return out

    # --- preselect top-C groups per query, rescore exactly ---
    C = max(2 * kk, kk + 64)
    part = np.argpartition(-vals, C, axis=1)[:, :C]
    vsel = np.take_along_axis(vals, part, 1)
    mem = members_of(part)                              # [B, C, FOLD]
    allq = np.arange(B)
    se = rescore(mem, allq)                             # [B, C, FOLD]
    gmax = se.max(2)
    finite = np.isfinite(gmax)
    delta = np.where(finite, np.abs(vsel - gmax), 0.0).max(1)
    margin = 4.0 * delta + 1e-3 * sigma

    flat = se.reshape(B, -1)
    vk = -np.partition(-flat, kk - 1, axis=1)[:, kk - 1]
    thr = vk - margin

    pool_v = [flat[q] for q in range(B)]
    pool_g = [mem[q].reshape(-1) for q in range(B)]

    # any group above thr that wasn't rescored yet
    selmask = np.zeros(vals.shape, dtype=bool)
    np.put_along_axis(selmask, part, True, 1)
    need = (vals >= thr[:, None]) & ~selmask
    for q in np.nonzero(need.any(1))[0]:
        g = np.nonzero(need[q])[0]
        m = members_of(g)[None]                          # [1, M, FOLD]
        sv = rescore(m, np.array([q]))[0]
        pool_v[q] = np.concatenate([pool_v[q], sv.reshape(-1)])
        pool_g[q] = np.concatenate([pool_g[q], m[0].reshape(-1)])

    # --- final exact top-k per query (dedupe, desc value, index tiebreak) --
    out_v = np.empty((B, kk), np.float32)
    out_g = np.empty((B, kk), np.int64)
    for q in range(B):
        keep = pool_g[q] >= 0
        g, first = np.unique(pool_g[q][keep], return_index=True)
        v32 = pool_v[q][keep][first].astype(np.float32)
        assert v32.size >= kk
        order = np.lexsort((g, -v32))[:kk]
        out_v[q] = v32[order]
        out_g[q] = g[order]

    top_ids = identifiers[out_g]
    return out_v, top_ids
